# revision 32
# baseline (speedup 1.0000x reference)
"""Causal attention (K Q^T variant) on 8 Trainium2 NeuronCores.

Problem: x[8,2048,1024], per-batch:
    Q = x@wq.T+bq; K = x@wk.T+bk; V = x@wv.T+bv
    S[t,s] = K[t]·Q[s]/sqrt(C), masked to s<=t, softmax over s
    out[t] = sum_s P[t,s] V[s]      -> [1,8,2048,1024] fp32

Sharding: data-parallel over batch B=8 across the 8 cores.

Algebraic reduction: S_raw[t,s] = x_t·G·x_s + a[t] + b[s] + c0 with
G = wk^T wq. a[t], c0 are constant along the softmax axis and drop; only
M = x G^T is computed on device and b[s]/sqrt(C) rides in the exp bias.

Precision/engine strategy:
  - The M projection and the scores GEMM run in fp8 e4m3 with
    MatmulPerfMode.DoubleRow (2 contraction chunks per instruction, ~2x
    PE throughput). G is pre-scaled by GS so its entries clear the e4m3
    denormal floor; the inverse scale folds into the exp's scale
    argument. Measured end-to-end rel err ~1.9e-2 against the fp32
    reference (gate: 2e-2); set MPROJ_FP8=False to drop back to
    ~1.3e-2 at ~15% more PE time.
  - V path stays fp16: V = x@wv^T (stationary x chunks) and the AV GEMM
    with a ones column producing the softmax denominator in the same
    PSUM accumulation.
  - Scores are computed transposed (S^T[s,t]) so the causal mask means
    above-diagonal tiles are skipped; the diagonal 128x128 block is
    masked by a 0/1 triangular multiply. Bounded scores -> exp without
    max subtraction.
  - AV blocks run in descending j so the final block (one i-group) makes
    a short kernel tail; its two column halves are split so normalize +
    store of half 0 overlap half 1's matmuls. Output is stored fp16 and
    widened to fp32 on the host.
"""

import numpy as np
import ml_dtypes

import concourse.mybir as mybir
import concourse.tile as tile
from concourse import bacc
from concourse.bass_utils import run_bass_kernel_spmd

P = 128
MMW = 512  # moving-operand slice width (one fp32 PSUM bank)
GS = 32.0  # host pre-scale on G (keeps e4m3 operands out of denormals)
MPROJ_FP8 = True

_BUILD_CACHE = {}


def build_attention_nc(T=2048, C=1024):
    key = (T, C)
    if key in _BUILD_CACHE:
        return _BUILD_CACHE[key]

    bf = mybir.dt.float16
    f8 = mybir.dt.float8e4
    f32 = mybir.dt.float32
    DR = mybir.MatmulPerfMode.DoubleRow
    NCC = C // P   # feature chunks (contraction)
    NT = T // P    # sequence chunks
    NJ = T // MMW  # moving slices per full row
    NH = C // MMW  # moving slices per V row
    VW = C + P     # V tile width incl. ones column at [C] plus pad
    SCALE = 1.0 / float(np.sqrt(np.float32(C)))

    nc = bacc.Bacc("TRN2", debug=False)
    xT = nc.dram_tensor("xT", [C, T], bf, kind="ExternalInput").ap()
    # x^T packed per contraction pair: x8[cp][p, k, t] = x^T[(2cp+k)*P+p, t]
    x8d = nc.dram_tensor("x8", [NCC // 2, P, 2, T], f8, kind="ExternalInput").ap()
    # G^T * GS pre-packed m-major: g8[m][p, c*P+w] = GS*G^T[c*P+p, m*P+w]
    g8d = nc.dram_tensor("g8", [NCC, P, C], f8, kind="ExternalInput").ap()
    # fp16 copy of the same packing (used only when MPROJ_FP8 is False)
    g16d = nc.dram_tensor("g16", [NCC, P, C], bf, kind="ExternalInput").ap()
    wvT = nc.dram_tensor("wvT", [C, C], bf, kind="ExternalInput").ap()
    bs2 = nc.dram_tensor("bs2", [P, NT], f32, kind="ExternalInput").ap()
    bvB = nc.dram_tensor("bvB", [P, C], f32, kind="ExternalInput").ap()
    out = nc.dram_tensor("out", [T, C], bf, kind="ExternalOutput").ap()

    AF = mybir.ActivationFunctionType
    EXP_SCALE = SCALE / GS

    with tile.TileContext(nc) as tc:
        with (
            tc.tile_pool(name="consts", bufs=1) as consts,
            tc.tile_pool(name="qkv", bufs=1) as qkv,
            tc.tile_pool(name="small", bufs=4) as small,
        ):
            bs_t = consts.tile([P, NT], f32, tag="bs")
            bvb = consts.tile([P, C], f32, tag="bvb")
            # tri[p, f] = 1.0 where p <= f else 0.0 (valid region of the
            # diagonal score block in [s-partition, t-free] coordinates)
            tri = consts.tile([P, P], bf, tag="tri")
            nc.gpsimd.memset(tri[:], 1.0)
            nc.gpsimd.affine_select(
                out=tri[:], in_=tri[:],
                compare_op=mybir.AluOpType.is_ge, fill=0.0,
                base=0, pattern=[[1, P]], channel_multiplier=-1,
            )

            x8 = qkv.tile([P, NCC, T], f8, tag="x8")
            x16 = qkv.tile([P, NCC, T], bf, tag="x16")
            MT = qkv.tile([P, NCC, T], f8, tag="MT")
            VA = qkv.tile([P, NT, VW], bf, tag="VA")
            PT = qkv.tile([P, NT, T], bf, tag="PT")

            with tc.tile_pool(name="xw", bufs=1) as xw:
                g8 = xw.tile([P, NCC, C], f8, tag="g8")
                wv_t = xw.tile([P, NCC, C], bf, tag="wv")
                xT_r = xT.rearrange("(c p) t -> p c t", p=P)
                wv_r = wvT.rearrange("(c p) o -> p c o", p=P)

                # All input DMAs go on the ONE sync queue in strict
                # consumption order: the hardware ring holds only ~7 transfers
                # in flight, so a single ordered queue acts as a priority
                # scheduler across the 16 DMA engines (~21 GB/s each, ~147
                # GB/s aggregate for the in-flight window). Pieces are
                # <=128KB so no engine is ever blocked long. Mproj consumes
                # j-major / m-outer, so: x8 j0 pieces, then g8 m-slices
                # (just-in-time, one per sweep step), then x8 j1..j3.
                g8_src = [g8d[m].rearrange("p (c w) -> p c w", w=P)
                          for m in range(NCC)]
                dmas = []  # (dst, src) in consumption order
                for cp in range(NCC // 2):
                    dmas.append((x8[:, 2 * cp:2 * cp + 2, 0:MMW],
                                 x8d[cp][:, :, 0:MMW]))
                for m in range(NCC):
                    dmas.append((g8[:, :, m * P:(m + 1) * P], g8_src[m]))
                for j in range(1, NJ):
                    for cp in range(NCC // 2):
                        dmas.append((x8[:, 2 * cp:2 * cp + 2,
                                        j * MMW:(j + 1) * MMW],
                                     x8d[cp][:, :, j * MMW:(j + 1) * MMW]))
                dmas.append((bs_t[:], bs2[:]))
                # x16 + wv for the later fp16 phases (the ring keeps these
                # from competing with the Mproj stream); c0/c1 go in halves
                # so Vproj's first stationary chunk is never the straggler
                for c in range(NCC):
                    if c < 2:
                        dmas.append((x16[:, c, 0:T // 2],
                                     xT_r[:, c, 0:T // 2]))
                        dmas.append((x16[:, c, T // 2:T],
                                     xT_r[:, c, T // 2:T]))
                    else:
                        dmas.append((x16[:, c, :], xT_r[:, c, :]))
                for c in range(NCC):
                    dmas.append((wv_t[:, c, :], wv_r[:, c, :]))
                dmas.append((bvb[:], bvB[:]))

                # Head of the stream alternates across two issue queues (the
                # ~0.65us per-issue cost would otherwise pace the ramp);
                # everything after rides the single ordered sync queue.
                HEAD = 0
                for k, (dst, src) in enumerate(dmas):
                    if k < HEAD and k % 2 == 1:
                        nc.gpsimd.dma_start(out=dst, in_=src)
                    else:
                        nc.sync.dma_start(out=dst, in_=src)

                # ---- M^T projection ----
                # fp8 DoubleRow, j-major: per j-slice all 8 m-groups run
                # interleaved on single-bank PSUM tiles, so each arriving x8
                # piece feeds 8 matmuls (keeps the DMA demand rate at
                # ~75 GB/s instead of ~300 during the startup ramp).
                if not MPROJ_FP8:
                    g16 = xw.tile([P, NCC, C], bf, tag="g16")
                    for m in range(NCC):
                        nc.sync.dma_start(
                            out=g16[:, :, m * P:(m + 1) * P],
                            in_=g16d[m].rearrange("p (c w) -> p c w", w=P),
                        )

                def mt_copy_part(m, j, pt):
                    # split the PSUM->SBUF casts across ScalarE/VectorE
                    dst = MT[:, m, j * MMW:(j + 1) * MMW]
                    if m % 2 == 0:
                        nc.scalar.copy(dst, pt[:])
                    else:
                        nc.vector.tensor_copy(dst, pt[:])

                with tc.tile_pool(name="psm", bufs=8, space="PSUM") as psm:
                    for j in range(NJ):
                        for m in range(NCC):
                            pt = psm.tile([P, MMW], f32, tag="psm",
                                          name=f"q{m}_{j}")
                            for cp in range(0, NCC, 2):
                                st = (cp == 0)
                                sp = (cp == NCC - 2)
                                if MPROJ_FP8:
                                    nc.tensor.matmul(
                                        pt[:],
                                        g8[:, cp:cp + 2, m * P:(m + 1) * P],
                                        x8[:, cp:cp + 2,
                                           j * MMW:(j + 1) * MMW],
                                        start=st, stop=sp, perf_mode=DR,
                                    )
                                else:
                                    for k in (0, 1):
                                        nc.tensor.matmul(
                                            pt[:],
                                            g16[:, cp + k,
                                                m * P:(m + 1) * P],
                                            x16[:, cp + k,
                                                j * MMW:(j + 1) * MMW],
                                            start=st and k == 0,
                                            stop=sp and k == 1,
                                        )
                            mt_copy_part(m, j, pt)

                # scores/Vproj PSUM pool (opened after psm closed so the two
                # never coexist; entered manually to keep indentation flat)
                ps_cm = tc.tile_pool(name="ps", bufs=2, space="PSUM")
                ps = ps_cm.__enter__()

                # ---- scores + exp ----
                # P^T chunk i covers t in [i*P, T); fp8 DoubleRow over
                # contraction pairs of M-feature chunks.
                def scores_chunk(i, pss=None, rebase=None):
                    if pss is None:
                        pss = ps.tile([P, T], f32, tag="ps", name="pss")
                    shift = 0 if rebase is None else rebase - i * P
                    jf = (i * P + MMW - 1) // MMW
                    slices = [(i * P, jf * MMW - i * P)] if i * P < jf * MMW else []
                    slices += [(j * MMW, MMW) for j in range(jf, NJ)]
                    for cp in range(0, NCC, 2):
                        st = (cp == 0)
                        sp = (cp == NCC - 2)
                        for (off, w) in slices:
                            nc.tensor.matmul(
                                pss[:, off + shift:off + shift + w],
                                MT[:, cp:cp + 2, i * P:(i + 1) * P],
                                x8[:, cp:cp + 2, off:off + w],
                                start=st, stop=sp, perf_mode=DR,
                            )
                    nc.scalar.activation(
                        PT[:, i, i * P:T],
                        pss[:, i * P + shift:T + shift], AF.Exp,
                        bias=bs_t[:, i:i + 1], scale=EXP_SCALE,
                    )
                    nc.vector.tensor_mul(
                        PT[:, i, i * P:(i + 1) * P],
                        PT[:, i, i * P:(i + 1) * P],
                        tri[:],
                    )
                    return pss

                for i in range(NT - 2):
                    scores_chunk(i)
                # last two (small) chunks share one tile in disjoint banks
                pss_tail = scores_chunk(NT - 2)
                scores_chunk(NT - 1, pss=pss_tail, rebase=0)

                # ---- V projection (fp16) ----
                # V[t-chunk n] = sum_c x^T[c][:, n-slice].T @ wv^T[c]
                for n in range(NT):
                    psv = ps.tile([P, C], f32, tag="ps")
                    for c in range(NCC):
                        for h in range(NH):
                            nc.tensor.matmul(
                                psv[:, h * MMW:(h + 1) * MMW],
                                x16[:, c, n * P:(n + 1) * P],
                                wv_t[:, c, h * MMW:(h + 1) * MMW],
                                start=(c == 0), stop=(c == NCC - 1),
                            )
                    nc.vector.tensor_add(VA[:, n, 0:C], psv[:, 0:C], bvb[:])
                    nc.vector.memset(VA[:, n, C:C + 1], 1.0)

                ps_cm.__exit__(None, None, None)

            # ---- AV + normalize (fp16), descending j for a short tail ----
            with (
                tc.tile_pool(name="outp", bufs=3) as outp,
                tc.tile_pool(name="ps2", bufs=2, space="PSUM") as ps2,
            ):

                def av_block(j, split_tail=False):
                    pso = ps2.tile([P, C + MMW], f32, tag="ps", name="pso")
                    if not split_tail:
                        for i in range(j + 1):
                            pt_s = PT[:, i, j * P:(j + 1) * P]
                            for h in range(NH):
                                nc.tensor.matmul(
                                    pso[:, h * MMW:(h + 1) * MMW],
                                    pt_s,
                                    VA[:, i, h * MMW:(h + 1) * MMW],
                                    start=(i == 0), stop=(i == j),
                                )
                            nc.tensor.matmul(
                                pso[:, C:C + 1],
                                pt_s,
                                VA[:, i, C:C + 1],
                                start=(i == 0), stop=(i == j),
                            )
                        rec = small.tile([P, 1], f32, tag="rec")
                        nc.vector.reciprocal(rec[:], pso[:, C:C + 1])
                        ot = outp.tile([P, C], bf, tag="ot")
                        nc.scalar.mul(ot[:], pso[:, 0:C], rec[:, 0:1])
                        nc.sync.dma_start(out=out[j * P:(j + 1) * P, :],
                                          in_=ot[:])
                        return
                    # split tail: pass 1 = half 0 + denominator
                    for i in range(j + 1):
                        pt_s = PT[:, i, j * P:(j + 1) * P]
                        nc.tensor.matmul(
                            pso[:, 0:MMW], pt_s, VA[:, i, 0:MMW],
                            start=(i == 0), stop=(i == j),
                        )
                        nc.tensor.matmul(
                            pso[:, C:C + 1], pt_s, VA[:, i, C:C + 1],
                            start=(i == 0), stop=(i == j),
                        )
                    rec = small.tile([P, 1], f32, tag="rec")
                    nc.vector.reciprocal(rec[:], pso[:, C:C + 1])
                    ot = outp.tile([P, C], bf, tag="ot")
                    nc.scalar.mul(ot[:, 0:MMW], pso[:, 0:MMW], rec[:, 0:1])
                    nc.sync.dma_start(out=out[j * P:(j + 1) * P, 0:MMW],
                                      in_=ot[:, 0:MMW])
                    # pass 2 on its own psum tile so its matmuls overlap
                    # pass 1's normalize + store
                    psoB = ps2.tile([P, MMW], f32, tag="psb", name="psoB")
                    for i in range(j + 1):
                        pt_s = PT[:, i, j * P:(j + 1) * P]
                        nc.tensor.matmul(
                            psoB[:], pt_s, VA[:, i, MMW:C],
                            start=(i == 0), stop=(i == j),
                        )
                    nc.scalar.mul(ot[:, MMW:C], psoB[:], rec[:, 0:1])
                    nc.sync.dma_start(out=out[j * P:(j + 1) * P, MMW:C],
                                      in_=ot[:, MMW:C])

                for j in range(NT - 1, 0, -1):
                    av_block(j)
                av_block(0, split_tail=(C > MMW))

    nc.compile()
    _BUILD_CACHE[key] = nc
    return nc


def make_in_maps(x, wq, bq, wk, bk, wv, bv):
    """Host-side shard + layout prep. One in_map per core (= batch element)."""
    f8 = ml_dtypes.float8_e4m3
    bfh = np.float16
    x = np.asarray(x, dtype=np.float32)
    B, T, C = x.shape
    NCC = C // P
    wq = np.asarray(wq, np.float32)
    wk = np.asarray(wk, np.float32)
    gTm = (wq.T @ wk) * np.float32(GS)            # [c_in(j), c_out(i)] * GS
    # m-major packing: g8[m][p, c*P+w] = gTm[c*P+p, m*P+w]
    gPk = np.ascontiguousarray(
        gTm.reshape(NCC, P, NCC, P).transpose(2, 1, 0, 3).reshape(NCC, P, C))
    g8 = gPk.astype(f8)
    g16 = gPk.astype(bfh)
    wvT = np.asarray(wv, np.float32).T.astype(bfh)
    v_b = wq.T @ np.asarray(bk, np.float32)       # [C]
    scale_div = np.float32(np.sqrt(np.float32(C)))
    bvf = np.ascontiguousarray(
        np.broadcast_to(np.asarray(bv, np.float32), (P, C)))
    in_maps = []
    for b in range(B):
        bs = (x[b] @ v_b) / scale_div             # [T] f32
        bs2 = np.ascontiguousarray(bs.reshape(T // P, P).T.astype(np.float32))
        xTb = np.ascontiguousarray(x[b].T)        # [C, T]
        # [cp][p, k, t] = xT[(2cp+k)*P+p, t]
        x8 = np.ascontiguousarray(
            xTb.reshape(NCC // 2, 2, P, T).transpose(0, 2, 1, 3)).astype(f8)
        in_maps.append({
            "xT": xTb.astype(bfh),
            "x8": x8, "g8": g8, "g16": g16, "wvT": wvT,
            "bs2": bs2, "bvB": bvf,
        })
    return in_maps


def kernel(x, wq, bq, wk, bk, wv, bv):
    x = np.asarray(x, dtype=np.float32)
    B, T, C = x.shape
    nc = build_attention_nc(T, C)
    in_maps = make_in_maps(x, wq, bq, wk, bk, wv, bv)
    res = run_bass_kernel_spmd(nc, in_maps, core_ids=list(range(B)))
    out = np.stack([res.results[b]["out"].astype(np.float32)
                    for b in range(B)], axis=0)[None]
    return np.ascontiguousarray(out)


# revision 33
# speedup vs baseline: 1.1852x; 1.1852x over previous
"""Causal attention (K Q^T variant) on 8 Trainium2 NeuronCores.

Problem: x[8,2048,1024], per-batch:
    Q = x@wq.T+bq; K = x@wk.T+bk; V = x@wv.T+bv
    S[t,s] = K[t]·Q[s]/sqrt(C), masked to s<=t, softmax over s
    out[t] = sum_s P[t,s] V[s]      -> [1,8,2048,1024] fp32

Sharding: data-parallel over batch B=8 across the 8 cores.

Algebraic reduction: S_raw[t,s] = x_t·G·x_s + a[t] + b[s] + c0 with
G = wk^T wq. a[t], c0 are constant along the softmax axis and drop; only
M = x G^T is computed on device and b[s]/sqrt(C) rides in the exp bias.

Precision/engine strategy:
  - The M projection and the scores GEMM run in fp8 e4m3 with
    MatmulPerfMode.DoubleRow (2 contraction chunks per instruction, ~2x
    PE throughput). G is pre-scaled by GS so its entries clear the e4m3
    denormal floor; the inverse scale folds into the exp's scale
    argument. Measured end-to-end rel err ~1.9e-2 against the fp32
    reference (gate: 2e-2); set MPROJ_FP8=False to drop back to
    ~1.3e-2 at ~15% more PE time.
  - V path stays fp16: V = x@wv^T (stationary x chunks) and the AV GEMM
    with a ones column producing the softmax denominator in the same
    PSUM accumulation.
  - Scores are computed transposed (S^T[s,t]) so the causal mask means
    above-diagonal tiles are skipped; the diagonal 128x128 block is
    masked by a 0/1 triangular multiply. Bounded scores -> exp without
    max subtraction.
  - AV blocks run in descending j so the final block (one i-group) makes
    a short kernel tail; its two column halves are split so normalize +
    store of half 0 overlap half 1's matmuls. Output is stored fp16 and
    widened to fp32 on the host.
"""

import numpy as np
import ml_dtypes

import concourse.mybir as mybir
import concourse.tile as tile
from concourse import bacc
from concourse.bass_utils import run_bass_kernel_spmd

P = 128
MMW = 512  # moving-operand slice width (one fp32 PSUM bank)
GS = 32.0  # host pre-scale on G (keeps e4m3 operands out of denormals)
MPROJ_FP8 = True

_BUILD_CACHE = {}


def build_attention_nc(T=2048, C=1024):
    key = (T, C)
    if key in _BUILD_CACHE:
        return _BUILD_CACHE[key]

    bf = mybir.dt.float16
    f8 = mybir.dt.float8e4
    f32 = mybir.dt.float32
    DR = mybir.MatmulPerfMode.DoubleRow
    NCC = C // P   # feature chunks (contraction)
    NT = T // P    # sequence chunks
    NJ = T // MMW  # moving slices per full row
    NH = C // MMW  # moving slices per V row
    VW = C + P     # V tile width incl. ones column at [C] plus pad
    SCALE = 1.0 / float(np.sqrt(np.float32(C)))

    nc = bacc.Bacc("TRN2", debug=False)
    xT = nc.dram_tensor("xT", [C, T], bf, kind="ExternalInput").ap()
    # x^T packed per contraction pair: x8[cp][p, k, t] = x^T[(2cp+k)*P+p, t]
    x8d = nc.dram_tensor("x8", [NCC // 2, P, 2, T], f8, kind="ExternalInput").ap()
    # G^T * GS pre-packed m-major: g8[m][p, c*P+w] = GS*G^T[c*P+p, m*P+w]
    g8d = nc.dram_tensor("g8", [NCC, P, C], f8, kind="ExternalInput").ap()
    # fp16 copy of the same packing (used only when MPROJ_FP8 is False)
    g16d = nc.dram_tensor("g16", [NCC, P, C], bf, kind="ExternalInput").ap()
    wvT = nc.dram_tensor("wvT", [C, C], bf, kind="ExternalInput").ap()
    bs2 = nc.dram_tensor("bs2", [P, NT], f32, kind="ExternalInput").ap()
    bvB = nc.dram_tensor("bvB", [P, C], f32, kind="ExternalInput").ap()
    out = nc.dram_tensor("out", [T, C], bf, kind="ExternalOutput").ap()

    AF = mybir.ActivationFunctionType
    EXP_SCALE = SCALE / GS

    with tile.TileContext(nc) as tc:
        with (
            tc.tile_pool(name="consts", bufs=1) as consts,
            tc.tile_pool(name="qkv", bufs=1) as qkv,
            tc.tile_pool(name="small", bufs=4) as small,
        ):
            bs_t = consts.tile([P, NT], f32, tag="bs")
            bvb = consts.tile([P, C], f32, tag="bvb")
            # tri[p, f] = 1.0 where p <= f else 0.0 (valid region of the
            # diagonal score block in [s-partition, t-free] coordinates)
            tri = consts.tile([P, P], bf, tag="tri")
            nc.gpsimd.memset(tri[:], 1.0)
            nc.gpsimd.affine_select(
                out=tri[:], in_=tri[:],
                compare_op=mybir.AluOpType.is_ge, fill=0.0,
                base=0, pattern=[[1, P]], channel_multiplier=-1,
            )

            x8 = qkv.tile([P, NCC, T], f8, tag="x8")
            x16 = qkv.tile([P, NCC, T], bf, tag="x16")
            MT = qkv.tile([P, NCC, T], f8, tag="MT")
            VA = qkv.tile([P, NT, VW], bf, tag="VA")
            PT = qkv.tile([P, NT, T], bf, tag="PT")

            with tc.tile_pool(name="xw", bufs=1) as xw:
                g8 = xw.tile([P, NCC, C], f8, tag="g8")
                wv_t = xw.tile([P, NCC, C], bf, tag="wv")
                xT_r = xT.rearrange("(c p) t -> p c t", p=P)
                wv_r = wvT.rearrange("(c p) o -> p c o", p=P)

                # All input DMAs go on the ONE sync queue in strict
                # consumption order: the hardware ring holds only ~7 transfers
                # in flight, so a single ordered queue acts as a priority
                # scheduler across the 16 DMA engines (~21 GB/s each, ~147
                # GB/s aggregate for the in-flight window). Pieces are
                # <=128KB so no engine is ever blocked long. Mproj consumes
                # j-major / m-outer, so: x8 j0 pieces, then g8 m-slices
                # (just-in-time, one per sweep step), then x8 j1..j3.
                g8_src = [g8d[m].rearrange("p (c w) -> p c w", w=P)
                          for m in range(NCC)]
                dmas = []  # (dst, src) in consumption order
                for cp in range(NCC // 2):
                    dmas.append((x8[:, 2 * cp:2 * cp + 2, 0:MMW],
                                 x8d[cp][:, :, 0:MMW]))
                for m in range(NCC):
                    dmas.append((g8[:, :, m * P:(m + 1) * P], g8_src[m]))
                for j in range(1, NJ):
                    for cp in range(NCC // 2):
                        dmas.append((x8[:, 2 * cp:2 * cp + 2,
                                        j * MMW:(j + 1) * MMW],
                                     x8d[cp][:, :, j * MMW:(j + 1) * MMW]))
                dmas.append((bs_t[:], bs2[:]))
                # x16 + wv for the later fp16 phases (the ring keeps these
                # from competing with the Mproj stream); c0/c1 go in halves
                # so Vproj's first stationary chunk is never the straggler
                for c in range(NCC):
                    if c < 2:
                        dmas.append((x16[:, c, 0:T // 2],
                                     xT_r[:, c, 0:T // 2]))
                        dmas.append((x16[:, c, T // 2:T],
                                     xT_r[:, c, T // 2:T]))
                    else:
                        dmas.append((x16[:, c, :], xT_r[:, c, :]))
                for c in range(NCC):
                    dmas.append((wv_t[:, c, :], wv_r[:, c, :]))
                dmas.append((bvb[:], bvB[:]))

                # Head of the stream alternates across two issue queues (the
                # ~0.65us per-issue cost would otherwise pace the ramp);
                # everything after rides the single ordered sync queue.
                HEAD = 8
                for k, (dst, src) in enumerate(dmas):
                    if k < HEAD and k % 2 == 1:
                        nc.gpsimd.dma_start(out=dst, in_=src)
                    else:
                        nc.sync.dma_start(out=dst, in_=src)

                # ---- M^T projection ----
                # fp8 DoubleRow, j-major: per j-slice all 8 m-groups run
                # interleaved on single-bank PSUM tiles, so each arriving x8
                # piece feeds 8 matmuls (keeps the DMA demand rate at
                # ~75 GB/s instead of ~300 during the startup ramp).
                if not MPROJ_FP8:
                    g16 = xw.tile([P, NCC, C], bf, tag="g16")
                    for m in range(NCC):
                        nc.sync.dma_start(
                            out=g16[:, :, m * P:(m + 1) * P],
                            in_=g16d[m].rearrange("p (c w) -> p c w", w=P),
                        )

                def mt_copy_part(m, j, pt):
                    # split the PSUM->SBUF casts across ScalarE/VectorE
                    dst = MT[:, m, j * MMW:(j + 1) * MMW]
                    if m % 2 == 0:
                        nc.scalar.copy(dst, pt[:])
                    else:
                        nc.vector.tensor_copy(dst, pt[:])

                with tc.tile_pool(name="psm", bufs=8, space="PSUM") as psm:
                    for j in range(NJ):
                        for m in range(NCC):
                            pt = psm.tile([P, MMW], f32, tag="psm",
                                          name=f"q{m}_{j}")
                            for cp in range(0, NCC, 2):
                                st = (cp == 0)
                                sp = (cp == NCC - 2)
                                if MPROJ_FP8:
                                    nc.tensor.matmul(
                                        pt[:],
                                        g8[:, cp:cp + 2, m * P:(m + 1) * P],
                                        x8[:, cp:cp + 2,
                                           j * MMW:(j + 1) * MMW],
                                        start=st, stop=sp, perf_mode=DR,
                                    )
                                else:
                                    for k in (0, 1):
                                        nc.tensor.matmul(
                                            pt[:],
                                            g16[:, cp + k,
                                                m * P:(m + 1) * P],
                                            x16[:, cp + k,
                                                j * MMW:(j + 1) * MMW],
                                            start=st and k == 0,
                                            stop=sp and k == 1,
                                        )
                            mt_copy_part(m, j, pt)

                # scores/Vproj PSUM pool (opened after psm closed so the two
                # never coexist; entered manually to keep indentation flat)
                ps_cm = tc.tile_pool(name="ps", bufs=2, space="PSUM")
                ps = ps_cm.__enter__()

                # ---- scores + exp ----
                # P^T chunk i covers t in [i*P, T); fp8 DoubleRow over
                # contraction pairs of M-feature chunks.
                def scores_chunk(i, pss=None, rebase=None):
                    if pss is None:
                        pss = ps.tile([P, T], f32, tag="ps", name="pss")
                    shift = 0 if rebase is None else rebase - i * P
                    jf = (i * P + MMW - 1) // MMW
                    slices = [(i * P, jf * MMW - i * P)] if i * P < jf * MMW else []
                    slices += [(j * MMW, MMW) for j in range(jf, NJ)]
                    for cp in range(0, NCC, 2):
                        st = (cp == 0)
                        sp = (cp == NCC - 2)
                        for (off, w) in slices:
                            nc.tensor.matmul(
                                pss[:, off + shift:off + shift + w],
                                MT[:, cp:cp + 2, i * P:(i + 1) * P],
                                x8[:, cp:cp + 2, off:off + w],
                                start=st, stop=sp, perf_mode=DR,
                            )
                    nc.scalar.activation(
                        PT[:, i, i * P:T],
                        pss[:, i * P + shift:T + shift], AF.Exp,
                        bias=bs_t[:, i:i + 1], scale=EXP_SCALE,
                    )
                    nc.vector.tensor_mul(
                        PT[:, i, i * P:(i + 1) * P],
                        PT[:, i, i * P:(i + 1) * P],
                        tri[:],
                    )
                    return pss

                for i in range(NT - 2):
                    scores_chunk(i)
                # last two (small) chunks share one tile in disjoint banks
                pss_tail = scores_chunk(NT - 2)
                scores_chunk(NT - 1, pss=pss_tail, rebase=0)

                # ---- V projection (fp16) ----
                # V[t-chunk n] = sum_c x^T[c][:, n-slice].T @ wv^T[c]
                for n in range(NT):
                    psv = ps.tile([P, C], f32, tag="ps")
                    for c in range(NCC):
                        for h in range(NH):
                            nc.tensor.matmul(
                                psv[:, h * MMW:(h + 1) * MMW],
                                x16[:, c, n * P:(n + 1) * P],
                                wv_t[:, c, h * MMW:(h + 1) * MMW],
                                start=(c == 0), stop=(c == NCC - 1),
                            )
                    nc.vector.tensor_add(VA[:, n, 0:C], psv[:, 0:C], bvb[:])
                    nc.vector.memset(VA[:, n, C:C + 1], 1.0)

                ps_cm.__exit__(None, None, None)

            # ---- AV + normalize (fp16), descending j for a short tail ----
            with (
                tc.tile_pool(name="outp", bufs=3) as outp,
                tc.tile_pool(name="ps2", bufs=2, space="PSUM") as ps2,
            ):

                def av_block(j, split_tail=False):
                    pso = ps2.tile([P, C + MMW], f32, tag="ps", name="pso")
                    if not split_tail:
                        for i in range(j + 1):
                            pt_s = PT[:, i, j * P:(j + 1) * P]
                            for h in range(NH):
                                nc.tensor.matmul(
                                    pso[:, h * MMW:(h + 1) * MMW],
                                    pt_s,
                                    VA[:, i, h * MMW:(h + 1) * MMW],
                                    start=(i == 0), stop=(i == j),
                                )
                            nc.tensor.matmul(
                                pso[:, C:C + 1],
                                pt_s,
                                VA[:, i, C:C + 1],
                                start=(i == 0), stop=(i == j),
                            )
                        rec = small.tile([P, 1], f32, tag="rec")
                        nc.vector.reciprocal(rec[:], pso[:, C:C + 1])
                        ot = outp.tile([P, C], bf, tag="ot")
                        nc.scalar.mul(ot[:], pso[:, 0:C], rec[:, 0:1])
                        nc.sync.dma_start(out=out[j * P:(j + 1) * P, :],
                                          in_=ot[:])
                        return
                    # split tail: pass 1 = half 0 + denominator
                    for i in range(j + 1):
                        pt_s = PT[:, i, j * P:(j + 1) * P]
                        nc.tensor.matmul(
                            pso[:, 0:MMW], pt_s, VA[:, i, 0:MMW],
                            start=(i == 0), stop=(i == j),
                        )
                        nc.tensor.matmul(
                            pso[:, C:C + 1], pt_s, VA[:, i, C:C + 1],
                            start=(i == 0), stop=(i == j),
                        )
                    rec = small.tile([P, 1], f32, tag="rec")
                    nc.vector.reciprocal(rec[:], pso[:, C:C + 1])
                    ot = outp.tile([P, C], bf, tag="ot")
                    nc.scalar.mul(ot[:, 0:MMW], pso[:, 0:MMW], rec[:, 0:1])
                    nc.sync.dma_start(out=out[j * P:(j + 1) * P, 0:MMW],
                                      in_=ot[:, 0:MMW])
                    # pass 2 on its own psum tile so its matmuls overlap
                    # pass 1's normalize + store
                    psoB = ps2.tile([P, MMW], f32, tag="psb", name="psoB")
                    for i in range(j + 1):
                        pt_s = PT[:, i, j * P:(j + 1) * P]
                        nc.tensor.matmul(
                            psoB[:], pt_s, VA[:, i, MMW:C],
                            start=(i == 0), stop=(i == j),
                        )
                    nc.scalar.mul(ot[:, MMW:C], psoB[:], rec[:, 0:1])
                    nc.sync.dma_start(out=out[j * P:(j + 1) * P, MMW:C],
                                      in_=ot[:, MMW:C])

                for j in range(NT - 1, 0, -1):
                    av_block(j)
                av_block(0, split_tail=(C > MMW))

    nc.compile()
    _BUILD_CACHE[key] = nc
    return nc


def make_in_maps(x, wq, bq, wk, bk, wv, bv):
    """Host-side shard + layout prep. One in_map per core (= batch element)."""
    f8 = ml_dtypes.float8_e4m3
    bfh = np.float16
    x = np.asarray(x, dtype=np.float32)
    B, T, C = x.shape
    NCC = C // P
    wq = np.asarray(wq, np.float32)
    wk = np.asarray(wk, np.float32)
    gTm = (wq.T @ wk) * np.float32(GS)            # [c_in(j), c_out(i)] * GS
    # m-major packing: g8[m][p, c*P+w] = gTm[c*P+p, m*P+w]
    gPk = np.ascontiguousarray(
        gTm.reshape(NCC, P, NCC, P).transpose(2, 1, 0, 3).reshape(NCC, P, C))
    g8 = gPk.astype(f8)
    g16 = gPk.astype(bfh)
    wvT = np.asarray(wv, np.float32).T.astype(bfh)
    v_b = wq.T @ np.asarray(bk, np.float32)       # [C]
    scale_div = np.float32(np.sqrt(np.float32(C)))
    bvf = np.ascontiguousarray(
        np.broadcast_to(np.asarray(bv, np.float32), (P, C)))
    in_maps = []
    for b in range(B):
        bs = (x[b] @ v_b) / scale_div             # [T] f32
        bs2 = np.ascontiguousarray(bs.reshape(T // P, P).T.astype(np.float32))
        xTb = np.ascontiguousarray(x[b].T)        # [C, T]
        # [cp][p, k, t] = xT[(2cp+k)*P+p, t]
        x8 = np.ascontiguousarray(
            xTb.reshape(NCC // 2, 2, P, T).transpose(0, 2, 1, 3)).astype(f8)
        in_maps.append({
            "xT": xTb.astype(bfh),
            "x8": x8, "g8": g8, "g16": g16, "wvT": wvT,
            "bs2": bs2, "bvB": bvf,
        })
    return in_maps


def kernel(x, wq, bq, wk, bk, wv, bv):
    x = np.asarray(x, dtype=np.float32)
    B, T, C = x.shape
    nc = build_attention_nc(T, C)
    in_maps = make_in_maps(x, wq, bq, wk, bk, wv, bv)
    res = run_bass_kernel_spmd(nc, in_maps, core_ids=list(range(B)))
    out = np.stack([res.results[b]["out"].astype(np.float32)
                    for b in range(B)], axis=0)[None]
    return np.ascontiguousarray(out)


# revision 35
# speedup vs baseline: 1.1902x; 1.0042x over previous
"""Causal attention (K Q^T variant) on 8 Trainium2 NeuronCores.

Problem: x[8,2048,1024], per-batch:
    Q = x@wq.T+bq; K = x@wk.T+bk; V = x@wv.T+bv
    S[t,s] = K[t]·Q[s]/sqrt(C), masked to s<=t, softmax over s
    out[t] = sum_s P[t,s] V[s]      -> [1,8,2048,1024] fp32

Sharding: data-parallel over batch B=8 across the 8 cores.

Algebraic reduction: S_raw[t,s] = x_t·G·x_s + a[t] + b[s] + c0 with
G = wk^T wq. a[t], c0 are constant along the softmax axis and drop; only
M = x G^T is computed on device and b[s]/sqrt(C) rides in the exp bias.

Precision/engine strategy:
  - The M projection and the scores GEMM run in fp8 e4m3 with
    MatmulPerfMode.DoubleRow (2 contraction chunks per instruction, ~2x
    PE throughput). G is pre-scaled by GS so its entries clear the e4m3
    denormal floor; the inverse scale folds into the exp's scale
    argument. Measured end-to-end rel err 1.83e-2 against the fp32
    reference (gate: 2e-2); set MPROJ_FP8=False to drop back to
    ~1.3e-2 at ~15% more PE time.
  - V path stays fp16: V = x@wv^T (stationary x chunks) and the AV GEMM
    with a ones column producing the softmax denominator in the same
    PSUM accumulation.
  - Scores are computed transposed (S^T[s,t]) so the causal mask means
    above-diagonal tiles are skipped; the diagonal 128x128 block is
    masked by a 0/1 triangular multiply. Bounded scores -> exp without
    max subtraction.
  - AV blocks run in descending j so the final block (one i-group) makes
    a short kernel tail; its two column halves are split so normalize +
    store of half 0 overlap half 1's matmuls. Output is stored fp16 and
    widened to fp32 on the host.
"""

import numpy as np
import ml_dtypes

import concourse.mybir as mybir
import concourse.tile as tile
from concourse import bacc
from concourse.bass_utils import run_bass_kernel_spmd

P = 128
MMW = 512  # moving-operand slice width (one fp32 PSUM bank)
GS = 32.0  # host pre-scale on G (keeps e4m3 operands out of denormals)
MPROJ_FP8 = True

_BUILD_CACHE = {}


def build_attention_nc(T=2048, C=1024):
    key = (T, C)
    if key in _BUILD_CACHE:
        return _BUILD_CACHE[key]

    bf = mybir.dt.float16
    f8 = mybir.dt.float8e4
    f32 = mybir.dt.float32
    DR = mybir.MatmulPerfMode.DoubleRow
    NCC = C // P   # feature chunks (contraction)
    NT = T // P    # sequence chunks
    NJ = T // MMW  # moving slices per full row
    NH = C // MMW  # moving slices per V row
    VW = C + P     # V tile width incl. ones column at [C] plus pad
    SCALE = 1.0 / float(np.sqrt(np.float32(C)))

    nc = bacc.Bacc("TRN2", debug=False)
    xT = nc.dram_tensor("xT", [C, T], bf, kind="ExternalInput").ap()
    # x^T packed per contraction pair: x8[cp][p, k, t] = x^T[(2cp+k)*P+p, t]
    x8d = nc.dram_tensor("x8", [NCC // 2, P, 2, T], f8, kind="ExternalInput").ap()
    # G^T * GS pre-packed m-major: g8[m][p, c*P+w] = GS*G^T[c*P+p, m*P+w]
    g8d = nc.dram_tensor("g8", [NCC, P, C], f8, kind="ExternalInput").ap()
    # fp16 copy of the same packing (used only when MPROJ_FP8 is False)
    g16d = nc.dram_tensor("g16", [NCC, P, C], bf, kind="ExternalInput").ap()
    wvT = nc.dram_tensor("wvT", [C, C], bf, kind="ExternalInput").ap()
    bs2 = nc.dram_tensor("bs2", [P, NT], f32, kind="ExternalInput").ap()
    bvB = nc.dram_tensor("bvB", [P, C], f32, kind="ExternalInput").ap()
    out = nc.dram_tensor("out", [T, C], bf, kind="ExternalOutput").ap()

    AF = mybir.ActivationFunctionType
    EXP_SCALE = SCALE / GS

    with tile.TileContext(nc) as tc:
        with (
            tc.tile_pool(name="consts", bufs=1) as consts,
            tc.tile_pool(name="qkv", bufs=1) as qkv,
            tc.tile_pool(name="small", bufs=4) as small,
        ):
            bs_t = consts.tile([P, NT], f32, tag="bs")
            bvb = consts.tile([P, C], f32, tag="bvb")
            # tri[p, f] = 1.0 where p <= f else 0.0 (valid region of the
            # diagonal score block in [s-partition, t-free] coordinates)
            tri = consts.tile([P, P], bf, tag="tri")
            nc.gpsimd.memset(tri[:], 1.0)
            nc.gpsimd.affine_select(
                out=tri[:], in_=tri[:],
                compare_op=mybir.AluOpType.is_ge, fill=0.0,
                base=0, pattern=[[1, P]], channel_multiplier=-1,
            )

            x8 = qkv.tile([P, NCC, T], f8, tag="x8")
            x16 = qkv.tile([P, NCC, T], bf, tag="x16")
            MT = qkv.tile([P, NCC, T], f8, tag="MT")
            VA = qkv.tile([P, NT, VW], bf, tag="VA")
            PT = qkv.tile([P, NT, T], bf, tag="PT")

            with tc.tile_pool(name="xw", bufs=1) as xw:
                g8 = xw.tile([P, NCC, C], f8, tag="g8")
                wv_t = xw.tile([P, NCC, C], bf, tag="wv")
                xT_r = xT.rearrange("(c p) t -> p c t", p=P)
                wv_r = wvT.rearrange("(c p) o -> p c o", p=P)

                # Input DMAs ride the sync queue in strict consumption
                # order: the hardware ring holds only ~7 transfers in
                # flight, so an ordered queue acts as a priority scheduler
                # across the 16 DMA engines (~21 GB/s each, ~147 GB/s
                # aggregate for the in-flight window). Pieces are <=128KB
                # so no engine is ever blocked long. Mproj consumes
                # j-major / m-outer, so: x8 j0 pieces, then g8 m-slices
                # (just-in-time, one per sweep step), then x8 j1..j3.
                g8_src = [g8d[m].rearrange("p (c w) -> p c w", w=P)
                          for m in range(NCC)]
                dmas = []  # (dst, src) in consumption order
                for cp in range(NCC // 2):
                    dmas.append((x8[:, 2 * cp:2 * cp + 2, 0:MMW],
                                 x8d[cp][:, :, 0:MMW]))
                for m in range(NCC):
                    dmas.append((g8[:, :, m * P:(m + 1) * P], g8_src[m]))
                for j in range(1, NJ):
                    for cp in range(NCC // 2):
                        dmas.append((x8[:, 2 * cp:2 * cp + 2,
                                        j * MMW:(j + 1) * MMW],
                                     x8d[cp][:, :, j * MMW:(j + 1) * MMW]))
                dmas.append((bs_t[:], bs2[:]))
                # x16 + wv for the later fp16 phases (the ring keeps these
                # from competing with the Mproj stream); c0/c1 go in halves
                # so Vproj's first stationary chunk is never the straggler
                for c in range(NCC):
                    if c < 2:
                        dmas.append((x16[:, c, 0:T // 2],
                                     xT_r[:, c, 0:T // 2]))
                        dmas.append((x16[:, c, T // 2:T],
                                     xT_r[:, c, T // 2:T]))
                    else:
                        dmas.append((x16[:, c, :], xT_r[:, c, :]))
                for c in range(NCC):
                    dmas.append((wv_t[:, c, :], wv_r[:, c, :]))
                dmas.append((bvb[:], bvB[:]))

                # Head of the stream alternates across two issue queues (the
                # ~0.65us per-issue cost would otherwise pace the ramp);
                # everything after rides the single ordered sync queue.
                HEAD = 8
                for k, (dst, src) in enumerate(dmas):
                    if k < HEAD and k % 2 == 1:
                        nc.gpsimd.dma_start(out=dst, in_=src)
                    else:
                        nc.sync.dma_start(out=dst, in_=src)

                # ---- M^T projection ----
                # fp8 DoubleRow, j-major: per j-slice all 8 m-groups run
                # interleaved on single-bank PSUM tiles, so each arriving x8
                # piece feeds 8 matmuls (keeps the DMA demand rate at
                # ~75 GB/s instead of ~300 during the startup ramp).
                if not MPROJ_FP8:
                    g16 = xw.tile([P, NCC, C], bf, tag="g16")
                    for m in range(NCC):
                        nc.sync.dma_start(
                            out=g16[:, :, m * P:(m + 1) * P],
                            in_=g16d[m].rearrange("p (c w) -> p c w", w=P),
                        )

                def mt_copy_part(m, j, pt):
                    # split the PSUM->SBUF casts across ScalarE/VectorE
                    dst = MT[:, m, j * MMW:(j + 1) * MMW]
                    if m % 2 == 0:
                        nc.scalar.copy(dst, pt[:])
                    else:
                        nc.vector.tensor_copy(dst, pt[:])

                with tc.tile_pool(name="psm", bufs=8, space="PSUM") as psm:
                    for j in range(NJ):
                        for m in range(NCC):
                            pt = psm.tile([P, MMW], f32, tag="psm",
                                          name=f"q{m}_{j}")
                            for cp in range(0, NCC, 2):
                                st = (cp == 0)
                                sp = (cp == NCC - 2)
                                if MPROJ_FP8:
                                    nc.tensor.matmul(
                                        pt[:],
                                        g8[:, cp:cp + 2, m * P:(m + 1) * P],
                                        x8[:, cp:cp + 2,
                                           j * MMW:(j + 1) * MMW],
                                        start=st, stop=sp, perf_mode=DR,
                                    )
                                else:
                                    for k in (0, 1):
                                        nc.tensor.matmul(
                                            pt[:],
                                            g16[:, cp + k,
                                                m * P:(m + 1) * P],
                                            x16[:, cp + k,
                                                j * MMW:(j + 1) * MMW],
                                            start=st and k == 0,
                                            stop=sp and k == 1,
                                        )
                            mt_copy_part(m, j, pt)

                # scores/Vproj PSUM pool (opened after psm closed so the two
                # never coexist; entered manually to keep indentation flat)
                ps_cm = tc.tile_pool(name="ps", bufs=2, space="PSUM")
                ps = ps_cm.__enter__()

                # ---- scores + exp ----
                # P^T chunk i covers t in [i*P, T); fp8 DoubleRow over
                # contraction pairs of M-feature chunks.
                def scores_chunk(i, pss=None, rebase=None):
                    if pss is None:
                        pss = ps.tile([P, T], f32, tag="ps", name="pss")
                    shift = 0 if rebase is None else rebase - i * P
                    jf = (i * P + MMW - 1) // MMW
                    slices = [(i * P, jf * MMW - i * P)] if i * P < jf * MMW else []
                    slices += [(j * MMW, MMW) for j in range(jf, NJ)]
                    for cp in range(0, NCC, 2):
                        st = (cp == 0)
                        sp = (cp == NCC - 2)
                        for (off, w) in slices:
                            nc.tensor.matmul(
                                pss[:, off + shift:off + shift + w],
                                MT[:, cp:cp + 2, i * P:(i + 1) * P],
                                x8[:, cp:cp + 2, off:off + w],
                                start=st, stop=sp, perf_mode=DR,
                            )
                    nc.scalar.activation(
                        PT[:, i, i * P:T],
                        pss[:, i * P + shift:T + shift], AF.Exp,
                        bias=bs_t[:, i:i + 1], scale=EXP_SCALE,
                    )
                    nc.vector.tensor_mul(
                        PT[:, i, i * P:(i + 1) * P],
                        PT[:, i, i * P:(i + 1) * P],
                        tri[:],
                    )
                    return pss

                for i in range(NT - 2):
                    scores_chunk(i)
                # last two (small) chunks share one tile in disjoint banks
                pss_tail = scores_chunk(NT - 2)
                scores_chunk(NT - 1, pss=pss_tail, rebase=0)

                # ---- V projection (fp16) ----
                # V[t-chunk n] = sum_c x^T[c][:, n-slice].T @ wv^T[c]
                for n in range(NT):
                    psv = ps.tile([P, C], f32, tag="ps")
                    for c in range(NCC):
                        for h in range(NH):
                            nc.tensor.matmul(
                                psv[:, h * MMW:(h + 1) * MMW],
                                x16[:, c, n * P:(n + 1) * P],
                                wv_t[:, c, h * MMW:(h + 1) * MMW],
                                start=(c == 0), stop=(c == NCC - 1),
                            )
                    nc.vector.tensor_add(VA[:, n, 0:C], psv[:, 0:C], bvb[:])
                    nc.vector.memset(VA[:, n, C:C + 1], 1.0)

                ps_cm.__exit__(None, None, None)

            # ---- AV + normalize (fp16), descending j for a short tail ----
            with (
                tc.tile_pool(name="outp", bufs=3) as outp,
                tc.tile_pool(name="ps2", bufs=2, space="PSUM") as ps2,
            ):

                def av_block(j, split_tail=False):
                    pso = ps2.tile([P, C + MMW], f32, tag="ps", name="pso")
                    if not split_tail:
                        for i in range(j + 1):
                            pt_s = PT[:, i, j * P:(j + 1) * P]
                            for h in range(NH):
                                nc.tensor.matmul(
                                    pso[:, h * MMW:(h + 1) * MMW],
                                    pt_s,
                                    VA[:, i, h * MMW:(h + 1) * MMW],
                                    start=(i == 0), stop=(i == j),
                                )
                            nc.tensor.matmul(
                                pso[:, C:C + 1],
                                pt_s,
                                VA[:, i, C:C + 1],
                                start=(i == 0), stop=(i == j),
                            )
                        rec = small.tile([P, 1], f32, tag="rec")
                        nc.vector.reciprocal(rec[:], pso[:, C:C + 1])
                        ot = outp.tile([P, C], bf, tag="ot")
                        nc.scalar.mul(ot[:], pso[:, 0:C], rec[:, 0:1])
                        nc.sync.dma_start(out=out[j * P:(j + 1) * P, :],
                                          in_=ot[:])
                        return
                    # split tail: pass 1 = half 0 + denominator
                    for i in range(j + 1):
                        pt_s = PT[:, i, j * P:(j + 1) * P]
                        nc.tensor.matmul(
                            pso[:, 0:MMW], pt_s, VA[:, i, 0:MMW],
                            start=(i == 0), stop=(i == j),
                        )
                        nc.tensor.matmul(
                            pso[:, C:C + 1], pt_s, VA[:, i, C:C + 1],
                            start=(i == 0), stop=(i == j),
                        )
                    rec = small.tile([P, 1], f32, tag="rec")
                    nc.vector.reciprocal(rec[:], pso[:, C:C + 1])
                    ot = outp.tile([P, C], bf, tag="ot")
                    nc.scalar.mul(ot[:, 0:MMW], pso[:, 0:MMW], rec[:, 0:1])
                    nc.sync.dma_start(out=out[j * P:(j + 1) * P, 0:MMW],
                                      in_=ot[:, 0:MMW])
                    # pass 2 on its own psum tile so its matmuls overlap
                    # pass 1's normalize + store
                    psoB = ps2.tile([P, MMW], f32, tag="psb", name="psoB")
                    for i in range(j + 1):
                        pt_s = PT[:, i, j * P:(j + 1) * P]
                        nc.tensor.matmul(
                            psoB[:], pt_s, VA[:, i, MMW:C],
                            start=(i == 0), stop=(i == j),
                        )
                    nc.scalar.mul(ot[:, MMW:C], psoB[:], rec[:, 0:1])
                    nc.sync.dma_start(out=out[j * P:(j + 1) * P, MMW:C],
                                      in_=ot[:, MMW:C])

                for j in range(NT - 1, 0, -1):
                    av_block(j)
                av_block(0, split_tail=(C > MMW))

    nc.compile()
    _BUILD_CACHE[key] = nc
    return nc


def make_in_maps(x, wq, bq, wk, bk, wv, bv):
    """Host-side shard + layout prep. One in_map per core (= batch element)."""
    f8 = ml_dtypes.float8_e4m3
    bfh = np.float16
    x = np.asarray(x, dtype=np.float32)
    B, T, C = x.shape
    NCC = C // P
    wq = np.asarray(wq, np.float32)
    wk = np.asarray(wk, np.float32)
    gTm = (wq.T @ wk) * np.float32(GS)            # [c_in(j), c_out(i)] * GS
    # m-major packing: g8[m][p, c*P+w] = gTm[c*P+p, m*P+w]
    gPk = np.ascontiguousarray(
        gTm.reshape(NCC, P, NCC, P).transpose(2, 1, 0, 3).reshape(NCC, P, C))
    g8 = gPk.astype(f8)
    g16 = gPk.astype(bfh)
    wvT = np.asarray(wv, np.float32).T.astype(bfh)
    v_b = wq.T @ np.asarray(bk, np.float32)       # [C]
    scale_div = np.float32(np.sqrt(np.float32(C)))
    bvf = np.ascontiguousarray(
        np.broadcast_to(np.asarray(bv, np.float32), (P, C)))
    in_maps = []
    for b in range(B):
        bs = (x[b] @ v_b) / scale_div             # [T] f32
        bs2 = np.ascontiguousarray(bs.reshape(T // P, P).T.astype(np.float32))
        xTb = np.ascontiguousarray(x[b].T)        # [C, T]
        # [cp][p, k, t] = xT[(2cp+k)*P+p, t]
        x8 = np.ascontiguousarray(
            xTb.reshape(NCC // 2, 2, P, T).transpose(0, 2, 1, 3)).astype(f8)
        in_maps.append({
            "xT": xTb.astype(bfh),
            "x8": x8, "g8": g8, "g16": g16, "wvT": wvT,
            "bs2": bs2, "bvB": bvf,
        })
    return in_maps


def kernel(x, wq, bq, wk, bk, wv, bv):
    x = np.asarray(x, dtype=np.float32)
    B, T, C = x.shape
    nc = build_attention_nc(T, C)
    in_maps = make_in_maps(x, wq, bq, wk, bk, wv, bv)
    res = run_bass_kernel_spmd(nc, in_maps, core_ids=list(range(B)))
    out = np.stack([res.results[b]["out"].astype(np.float32)
                    for b in range(B)], axis=0)[None]
    return np.ascontiguousarray(out)
